# revision 19
# baseline (speedup 1.0000x reference)
"""Trainium2 Bass kernel for nn_AttentionBlock (B=32,S=512,D=768,H=12,FF=3072).

Strategy: pure data-parallel over batch. 8 NeuronCores x 4 batches each
(2048 tokens/core). Each core runs an identical Tile program:

  LN1 (token-major, per-partition stats) -> PE-transpose -> xn^T (feature-major)
  QKV: q^T,k^T feature-major; v token-major (with ones column for softmax sums)
  attention per (h,b): S^T = k^T.T @ q^T (transposed scores), P = exp(S^T)*exp(bias^T)
     (exp(edge_bias) precomputed on host; no max-subtraction needed: scores are tiny)
  o token-major = P^T.T @ [v|1]; per-partition softmax normalization
  o -> PE-transpose -> o^T -> proj -> transpose -> +x residual (fp32)
  LN2 -> transpose -> FFN (fc1+gelu -> h^T resident -> fc2) -> transpose -> +x2 -> out

All matmuls in bf16 (fp32 PSUM accumulation); LN stats, softmax denominators and
the residual stream stay fp32. LN gains/biases and the q-scale are folded into
the weights on the host; weights are pre-tiled on the host so every DMA is a
contiguous block.
"""

import numpy as np
import ml_dtypes

import concourse.bass as bass
import concourse.mybir as mybir
import concourse.tile as tile
from concourse import bacc
from concourse.bass_utils import run_bass_kernel_spmd
from concourse.masks import make_identity

AF = mybir.ActivationFunctionType
ALU = mybir.AluOpType
F32 = mybir.dt.float32
BF16 = mybir.dt.bfloat16

B, S, D, H = 32, 512, 768, 12
HD = D // H           # 64
FF = 4 * D            # 3072
EPS = 1e-5
N_CORES = 8
BPC = B // N_CORES    # 4 batches per core
T = BPC * S           # 2048 tokens per core
TT = T // 128         # 16 token tiles
DT = D // 128         # 6 feature tiles
FFT = FF // 128       # 24 ffn tiles
NCH = T // 512        # 4 column chunks of 512 tokens

bf16 = ml_dtypes.bfloat16


def _prep_weights(edge_bias, ln1_g, ln1_b, qkv_w, qkv_b, proj_w, proj_b,
                  ln2_g, ln2_b, fc1_w, fc1_b, fc2_w, fc2_b):
    """Host-side folding + retiling. Returns dict of np arrays for in_maps."""
    f32 = np.float32
    qkv_w = np.asarray(qkv_w, f32)
    qkv_b = np.asarray(qkv_b, f32)
    # fold LN1 affine into qkv
    w_eff = qkv_w * np.asarray(ln1_g, f32)[None, :]
    b_eff = qkv_b + qkv_w @ np.asarray(ln1_b, f32)
    # fold q scaling
    scale = HD ** -0.5
    w_eff[:D] *= scale
    b_eff = b_eff.copy()
    b_eff[:D] *= scale

    # q/k lhsT blocks: [12 fblocks, 128 dpart, 6 dt, 128 f] -> [12, 128, 768]
    wqk = w_eff[:2 * D].reshape(2 * DT, 128, DT, 128).transpose(0, 3, 2, 1)
    wqk = np.ascontiguousarray(wqk.reshape(2 * DT, 128, D), dtype=bf16)
    bqk = np.ascontiguousarray(b_eff[:2 * D].reshape(2 * DT, 128).T, dtype=f32)  # [128, 12]

    # v rhs blocks: [128 dpart, 6 dt, 768 vf] -> [128, 4608]
    wv = w_eff[2 * D:].reshape(D, DT, 128).transpose(2, 1, 0)
    wv = np.ascontiguousarray(wv.reshape(128, DT * D), dtype=bf16)
    bv = np.ascontiguousarray(b_eff[2 * D:].reshape(1, D), dtype=bf16)

    # proj lhsT blocks: [6 gblocks, 128 fpart, 6 ft, 128 g] -> [6, 128, 768]
    proj_w = np.asarray(proj_w, f32)
    wp = proj_w.reshape(DT, 128, DT, 128).transpose(0, 3, 2, 1)
    wp = np.ascontiguousarray(wp.reshape(DT, 128, D), dtype=bf16)
    bp = np.ascontiguousarray(np.asarray(proj_b, f32).reshape(DT, 128).T, dtype=f32)  # [128, 6]

    # fc1 (fold LN2): [24 ffblocks, 128 dpart, 6 dt, 128 ff] -> [24, 128, 768]
    fc1_w = np.asarray(fc1_w, f32)
    w1_eff = fc1_w * np.asarray(ln2_g, f32)[None, :]
    b1_eff = np.asarray(fc1_b, f32) + fc1_w @ np.asarray(ln2_b, f32)
    w1 = w1_eff.reshape(FFT, 128, DT, 128).transpose(0, 3, 2, 1)
    w1 = np.ascontiguousarray(w1.reshape(FFT, 128, D), dtype=bf16)
    b1 = np.ascontiguousarray(b1_eff.reshape(FFT, 128).T, dtype=f32)  # [128, 24]

    # fc2: [6 gblocks, 128 ffpart, 24 fft, 128 g] -> [6, 128, 3072]
    fc2_w = np.asarray(fc2_w, f32)
    w2 = fc2_w.reshape(DT, 128, FFT, 128).transpose(0, 3, 2, 1)
    w2 = np.ascontiguousarray(w2.reshape(DT, 128, FF), dtype=bf16)
    b2 = np.ascontiguousarray(np.asarray(fc2_b, f32).reshape(DT, 128).T, dtype=f32)  # [128, 6]

    # exp(edge_bias) transposed: [12, 128 ktpart, 4 kt, 512 qt] -> [12, 128, 2048]
    eb = np.exp(np.asarray(edge_bias, f32)).transpose(0, 2, 1)  # [H, kt, qt]
    eb = eb.reshape(H, NCH, 128, S).transpose(0, 2, 1, 3)
    eb = np.ascontiguousarray(eb.reshape(H, 128, NCH * S), dtype=bf16)

    return dict(wqk=wqk, bqk=bqk, wv=wv, bv=bv, wp=wp, bp=bp,
                w1=w1, b1=b1, w2=w2, b2=b2, expb=eb)


def _layernorm_tiles(nc, sb_small, x_ap, out_ap, apply_on_act=False):
    """Token-major LN core: out (bf16) = (x - mean)/sqrt(var+EPS).

    Stats on DVE (bn_stats/bn_aggr); apply is a single fused DVE tensor_scalar:
    (x + (-mean)) * rinv.
    """
    bn = sb_small.tile([128, 2, 6], F32, tag="bn")
    nc.vector.bn_stats(bn[:, 0, :], x_ap[:, 0:512])
    nc.vector.bn_stats(bn[:, 1, :], x_ap[:, 512:768])
    mv = sb_small.tile([128, 2], F32, tag="mv")
    nc.vector.bn_aggr(mv[:], bn[:])
    std = sb_small.tile([128, 1], F32, tag="std")
    nc.scalar.activation(std[:], mv[:, 1:2], AF.Sqrt, bias=EPS)
    rinv = sb_small.tile([128, 1], F32, tag="rinv")
    nc.vector.reciprocal(rinv[:], std[:])
    negmean = sb_small.tile([128, 1], F32, tag="negmean")
    nc.vector.tensor_scalar(negmean[:], mv[:, 0:1], -1.0, None, op0=ALU.mult)
    if apply_on_act:
        negmur = sb_small.tile([128, 1], F32, tag="negmur")
        nc.vector.tensor_tensor(negmur[:], negmean[:], rinv[:], op=ALU.mult)
        nc.scalar.activation(out_ap, x_ap, AF.Identity, bias=negmur[:], scale=rinv[:])
    else:
        nc.vector.tensor_scalar(out_ap, x_ap, negmean[:], rinv[:],
                                op0=ALU.add, op1=ALU.mult)


def build_program(reps=1, upto=6):
    nc = bacc.Bacc("TRN2", target_bir_lowering=False, debug=False)

    x = nc.dram_tensor("x", [T, D], F32, kind="ExternalInput").ap()
    wqk = nc.dram_tensor("wqk", [2 * DT, 128, D], BF16, kind="ExternalInput").ap()
    bqk = nc.dram_tensor("bqk", [128, 2 * DT], F32, kind="ExternalInput").ap()
    wv = nc.dram_tensor("wv", [128, DT * D], BF16, kind="ExternalInput").ap()
    bv = nc.dram_tensor("bv", [1, D], BF16, kind="ExternalInput").ap()
    wp = nc.dram_tensor("wp", [DT, 128, D], BF16, kind="ExternalInput").ap()
    bp = nc.dram_tensor("bp", [128, DT], F32, kind="ExternalInput").ap()
    w1 = nc.dram_tensor("w1", [FFT, 128, D], BF16, kind="ExternalInput").ap()
    b1 = nc.dram_tensor("b1", [128, FFT], F32, kind="ExternalInput").ap()
    w2 = nc.dram_tensor("w2", [DT, 128, FF], BF16, kind="ExternalInput").ap()
    b2 = nc.dram_tensor("b2", [128, DT], F32, kind="ExternalInput").ap()
    expb = nc.dram_tensor("expb", [H, 128, NCH * S], BF16, kind="ExternalInput").ap()
    out = nc.dram_tensor("out", [T, D], F32, kind="ExternalOutput").ap()

    with tile.TileContext(nc) as tc:
        with (
            tc.tile_pool(name="const", bufs=1) as constp,
            tc.tile_pool(name="small", bufs=4) as sb_small,
        ):
            ident = constp.tile([128, 128], BF16)
            make_identity(nc, ident[:])
            ones1 = constp.tile([1, 128], BF16)
            nc.vector.memset(ones1[:], 1.0)
            zero_c = constp.tile([128, 1], F32)
            nc.vector.memset(zero_c[:], 0.0)
            nc.const_aps.aps[(F32, 0.0)] = zero_c[:]
            eps_c = constp.tile([128, 1], F32)
            nc.vector.memset(eps_c[:], EPS)
            nc.const_aps.aps[(F32, EPS)] = eps_c[:]

            bqk_sb = constp.tile([128, 2 * DT], F32)
            nc.sync.dma_start(bqk_sb[:], bqk)
            bv_sb = constp.tile([1, D], BF16)
            nc.sync.dma_start(bv_sb[:], bv)
            bp_sb = constp.tile([128, DT], F32)
            nc.sync.dma_start(bp_sb[:], bp)
            b1_sb = constp.tile([128, FFT], F32)
            nc.sync.dma_start(b1_sb[:], b1)
            b2_sb = constp.tile([128, DT], F32)
            nc.sync.dma_start(b2_sb[:], b2)

            open_pools = []

            def pool(**kw):
                p = tc.alloc_tile_pool(**kw)
                open_pools.append(p)
                return p

            def rel(p):
                p.release()
                open_pools.remove(p)

            for _rep in range(reps):
                # ---------- Phase 1: LN1 + transpose -> xnT ----------
                pool_x = pool(name="xres", bufs=1)
                x_res = pool_x.tile([128, TT, D], F32)  # resident x, 48KB/part
                pool_xnT = pool(name="xnT", bufs=1)
                xnT = pool_xnT.tile([128, DT, T], BF16)  # 24KB/part
                with (
                    tc.tile_pool(name="p1", bufs=3) as p1,
                    tc.tile_pool(name="p1ps", bufs=2, space="PSUM") as p1ps,
                ):
                    for it in range(TT):
                        nc.sync.dma_start(x_res[:, it, :], x[it * 128:(it + 1) * 128, :])
                        xn_t = p1.tile([128, D], BF16, tag="xn")
                        _layernorm_tiles(nc, sb_small, x_res[:, it, :], xn_t[:])
                        tp = p1ps.tile([128, DT, 128], BF16, tag="tp")
                        for dt in range(DT):
                            nc.tensor.transpose(tp[:, dt, :], xn_t[:, dt * 128:(dt + 1) * 128], ident[:])
                        nc.vector.tensor_copy(xnT[:, :, it * 128:(it + 1) * 128], tp[:])

                # ---------- Phase 2+3 merged: QKV + attention ----------
                # v first; then per qk f-block pair (q_fb, k_fb) compute the
                # two heads' attention immediately — ACT exp and DVE softmax
                # work hides under the PE matmul stream.
                if upto >= 2:
                    pool_v = pool(name="vsb", bufs=1)
                    v_sb = pool_v.tile([128, TT, H, HD + 1], BF16)
                    nc.vector.memset(v_sb[:, :, :, HD], 1.0)
                    with (
                        tc.tile_pool(name="p2wv", bufs=1) as p2wv,
                        tc.tile_pool(name="p2vps", bufs=2, space="PSUM") as p2vps,
                    ):
                        wv_sb = p2wv.tile([128, DT, D], BF16, tag="wv")
                        nc.sync.dma_start(wv_sb[:], wv.rearrange("p (k f) -> p k f", k=DT))
                        for it in range(TT):
                            vps = p2vps.tile([128, D], F32, tag="vps")  # 2 banks
                            for kt in range(DT):
                                nc.tensor.matmul(vps[:, 0:512], xnT[:, kt, it * 128:(it + 1) * 128],
                                                 wv_sb[:, kt, 0:512], start=(kt == 0), stop=False)
                                nc.tensor.matmul(vps[:, 512:D], xnT[:, kt, it * 128:(it + 1) * 128],
                                                 wv_sb[:, kt, 512:D], start=(kt == 0), stop=False)
                            nc.tensor.matmul(vps[:, 0:512], ones1[:], bv_sb[:, 0:512],
                                             start=False, stop=True)
                            nc.tensor.matmul(vps[:, 512:D], ones1[:], bv_sb[:, 512:D],
                                             start=False, stop=True)
                            nc.scalar.activation(v_sb[:, it, :, 0:HD],
                                                 vps[:].rearrange("p (h d) -> p h d", h=H),
                                                 AF.Identity)

                    if upto >= 3:
                        pool_o = pool(name="o", bufs=1, side="right")
                        o_sb = pool_o.tile([128, TT, H, HD], BF16)  # 24KB/part
                    with (
                        tc.tile_pool(name="p2w", bufs=3) as p2w,
                        tc.tile_pool(name="p23qk", bufs=2) as p23qk,
                        tc.tile_pool(name="p3eb", bufs=3) as p3eb,
                        tc.tile_pool(name="p3p", bufs=3) as p3p,
                        tc.tile_pool(name="p2ps", bufs=1, space="PSUM") as p2ps,
                        tc.tile_pool(name="p3sps", bufs=2, space="PSUM") as p3sps,
                        tc.tile_pool(name="p3ops", bufs=2, space="PSUM") as p3ops,
                    ):
                        for fp in range(DT):
                            # q f-block (fp) and k f-block (DT+fp) -> one pair tile
                            qk_t = p23qk.tile([128, 2, T], BF16, tag="qkpair")
                            for which, fb in ((0, fp), (1, DT + fp)):
                                wqk_t = p2w.tile([128, DT, 128], BF16, tag="wqk")
                                nc.sync.dma_start(wqk_t[:], wqk[fb].rearrange("p (k f) -> p k f", k=DT))
                                for chh in range(2):
                                    ps = p2ps.tile([128, T // 2], F32, tag="qkps")  # 2 banks
                                    for kt in range(DT):
                                        for ch in range(2):
                                            c = chh * 2 + ch
                                            nc.tensor.matmul(ps[:, ch * 512:(ch + 1) * 512],
                                                             wqk_t[:, kt, :],
                                                             xnT[:, kt, c * 512:(c + 1) * 512],
                                                             start=(kt == 0), stop=(kt == DT - 1))
                                    nc.scalar.activation(
                                        qk_t[:, which, chh * 1024:(chh + 1) * 1024],
                                        ps[:], AF.Identity, bias=bqk_sb[:, fb:fb + 1])
                            if upto < 3:
                                continue
                            # both heads of this f-block pair, scores emitted
                            # adjacently in disjoint PE row-groups (0-63 / 64-127)
                            ebs = []
                            for h in (2 * fp, 2 * fp + 1):
                                eb_tile = p3eb.tile([128, NCH, S], BF16, tag="eb")
                                nc.sync.dma_start(eb_tile[:],
                                                  expb[h].rearrange("p (j q) -> p j q", j=NCH))
                                ebs.append(eb_tile)
                            for b in range(BPC):
                                pTs = []
                                for hh in range(2):
                                    pT_h = p3p.tile([128, NCH, S], BF16, tag="pT")
                                    pTs.append(pT_h)
                                for jj in range(NCH // 2):
                                    sp0 = p3sps.tile([128, 2, S], F32, tag="s")
                                    sp1 = p3sps.tile([128, 2, S], F32, tag="s")
                                    for sub in range(2):
                                        j = jj * 2 + sub
                                        for hh, sp in ((0, sp0), (1, sp1)):
                                            prow = hh * HD
                                            q_ap = qk_t[prow:prow + HD, 0, b * S:(b + 1) * S]
                                            k_ap = qk_t[prow:prow + HD, 1,
                                                        b * S + j * 128: b * S + (j + 1) * 128]
                                            nc.tensor.matmul(sp[:, sub, :], k_ap, q_ap,
                                                             start=True, stop=True,
                                                             tile_position=(prow, 0))
                                    for hh, sp in ((0, sp0), (1, sp1)):
                                        nc.scalar.activation(
                                            pTs[hh][:, 2 * jj:2 * jj + 2, :], sp[:], AF.Exp)
                                        nc.vector.tensor_tensor(
                                            pTs[hh][:, 2 * jj:2 * jj + 2, :],
                                            pTs[hh][:, 2 * jj:2 * jj + 2, :],
                                            ebs[hh][:, 2 * jj:2 * jj + 2, :], op=ALU.mult)
                                for hh in range(2):
                                    h = 2 * fp + hh
                                    pT = pTs[hh]
                                    ops = p3ops.tile([128, NCH, HD + 1], F32, tag="o")
                                    for qc in range(NCH):
                                        for j in range(NCH):
                                            nc.tensor.matmul(ops[:, qc, :],
                                                             pT[:, j, qc * 128:(qc + 1) * 128],
                                                             v_sb[:, b * NCH + j, h, :],
                                                             start=(j == 0), stop=(j == NCH - 1))
                                    rec = sb_small.tile([128, NCH], F32, tag="rec")
                                    nc.vector.reciprocal(rec[:], ops[:, :, HD])
                                    for qc in range(NCH):
                                        nc.vector.tensor_scalar(o_sb[:, b * NCH + qc, h, :],
                                                                ops[:, qc, 0:HD],
                                                                rec[:, qc:qc + 1], None, op0=ALU.mult)
                    rel(pool_v)
                    rel(pool_xnT)

                # ---------- Phase 4: o^T, proj, residual -> x2 ----------
                if upto >= 4:
                    pool_p4 = pool(name="p4big", bufs=1)
                    with tc.tile_pool(name="p4", bufs=3) as p4:
                        oT = pool_p4.tile([128, DT, T], BF16, tag="oT")  # 24KB/part
                        with tc.tile_pool(name="p4ps", bufs=2, space="PSUM") as p4ps:
                            for it in range(TT):
                                tp = p4ps.tile([128, DT, 128], BF16, tag="otp")
                                o_flat = o_sb[:, it, :, :].rearrange("p h d -> p (h d)")
                                for dt in range(DT):
                                    nc.tensor.transpose(tp[:, dt, :],
                                                        o_flat[:, dt * 128:(dt + 1) * 128], ident[:])
                                nc.vector.tensor_copy(oT[:, :, it * 128:(it + 1) * 128], tp[:])
                        rel(pool_o)
                        pool_x2 = pool(name="x2", bufs=1, side="right")
                        x2_sb = pool_x2.tile([128, TT, D], F32)  # 48KB/part
                        pjT = pool_p4.tile([128, DT, T], BF16, tag="pjT")
                        with tc.tile_pool(name="p4pps", bufs=2, space="PSUM") as p4pps:
                            for gb in range(DT):
                                wp_t = p4.tile([128, DT, 128], BF16, tag="wp")
                                nc.sync.dma_start(wp_t[:], wp[gb].rearrange("p (k f) -> p k f", k=DT))
                                pps = p4pps.tile([128, T], F32, tag="pjps")
                                for kt in range(DT):
                                    for ch in range(NCH):
                                        nc.tensor.matmul(pps[:, ch * 512:(ch + 1) * 512],
                                                         wp_t[:, kt, :],
                                                         oT[:, kt, ch * 512:(ch + 1) * 512],
                                                         start=(kt == 0), stop=(kt == DT - 1))
                                nc.scalar.activation(pjT[:, gb, :], pps[:], AF.Identity,
                                                     bias=bp_sb[:, gb:gb + 1])
                        with tc.tile_pool(name="p4tps", bufs=2, space="PSUM") as p4tps:
                            for it in range(TT):
                                tp = p4tps.tile([128, DT, 128], BF16, tag="rtp")
                                for dt in range(DT):
                                    nc.tensor.transpose(tp[:, dt, :],
                                                        pjT[:, dt, it * 128:(it + 1) * 128], ident[:])
                                nc.vector.tensor_tensor(x2_sb[:, it, :], x_res[:, it, :],
                                                        tp[:].rearrange("p k f -> p (k f)"), op=ALU.add)
                    rel(pool_p4)
                    rel(pool_x)

                # ---------- Phase 5: LN2 -> xn2T ----------
                if upto >= 5:
                    pool_xn2 = pool(name="xn2", bufs=1, side="right")
                    xn2T = pool_xn2.tile([128, DT, T], BF16)
                    with tc.tile_pool(name="p5", bufs=3) as p5, \
                         tc.tile_pool(name="p5ps", bufs=2, space="PSUM") as p5ps:
                        for it in range(TT):
                            xn2_t = p5.tile([128, D], BF16, tag="xn2")
                            _layernorm_tiles(nc, sb_small, x2_sb[:, it, :], xn2_t[:], apply_on_act=True)
                            tp = p5ps.tile([128, DT, 128], BF16, tag="tp2")
                            for dt in range(DT):
                                nc.tensor.transpose(tp[:, dt, :],
                                                    xn2_t[:, dt * 128:(dt + 1) * 128], ident[:])
                            nc.vector.tensor_copy(xn2T[:, :, it * 128:(it + 1) * 128], tp[:])

                # ---------- Phase 6: FFN + final residual ----------
                if upto >= 6:
                    pool_h = pool(name="hsb", bufs=1)
                    h_sb = pool_h.tile([128, FFT, T], BF16)  # 96KB/part
                    with tc.tile_pool(name="p6w1", bufs=2) as p6w1, \
                         tc.tile_pool(name="p6ps", bufs=2, space="PSUM") as p6ps:
                        for ffb in range(FFT):
                            w1_t = p6w1.tile([128, DT, 128], BF16, tag="w1")
                            nc.sync.dma_start(w1_t[:], w1[ffb].rearrange("p (k f) -> p k f", k=DT))
                            hps = p6ps.tile([128, T], F32, tag="hps")
                            for kt in range(DT):
                                for ch in range(NCH):
                                    nc.tensor.matmul(hps[:, ch * 512:(ch + 1) * 512],
                                                     w1_t[:, kt, :],
                                                     xn2T[:, kt, ch * 512:(ch + 1) * 512],
                                                     start=(kt == 0), stop=(kt == DT - 1))
                            nc.scalar.activation(h_sb[:, ffb, :], hps[:], AF.Gelu,
                                                 bias=b1_sb[:, ffb:ffb + 1])
                    rel(pool_xn2)

                    with (
                        tc.tile_pool(name="p6f", bufs=1) as p6f,
                        tc.tile_pool(name="p6w2", bufs=2) as p6w2,
                        tc.tile_pool(name="p6o", bufs=3) as p6o,
                    ):
                        f2T = p6f.tile([128, DT, T], BF16, tag="f2T")
                        # fc2 in 1024-wide halves, double-buffered psum
                        with tc.tile_pool(name="p6f2ps", bufs=2, space="PSUM") as p6f2ps:
                            for gb in range(DT):
                                w2_t = p6w2.tile([128, FFT, 128], BF16, tag="w2")
                                nc.sync.dma_start(w2_t[:], w2[gb].rearrange("p (k f) -> p k f", k=FFT))
                                for half in range(2):
                                    fps = p6f2ps.tile([128, T // 2], F32, tag="f2ps")  # 2 banks
                                    for kt in range(FFT):
                                        for ch in range(2):
                                            c = half * 2 + ch
                                            nc.tensor.matmul(fps[:, ch * 512:(ch + 1) * 512],
                                                             w2_t[:, kt, :],
                                                             h_sb[:, kt, c * 512:(c + 1) * 512],
                                                             start=(kt == 0), stop=(kt == FFT - 1))
                                    nc.scalar.activation(f2T[:, gb, half * 1024:(half + 1) * 1024],
                                                         fps[:], AF.Identity,
                                                         bias=b2_sb[:, gb:gb + 1])
                        with tc.tile_pool(name="p6tps", bufs=2, space="PSUM") as p6tps:
                            for it in range(TT):
                                tp = p6tps.tile([128, DT, 128], BF16, tag="ftp")
                                for dt in range(DT):
                                    nc.tensor.transpose(tp[:, dt, :],
                                                        f2T[:, dt, it * 128:(it + 1) * 128], ident[:])
                                out_t = p6o.tile([128, D], F32, tag="out")
                                nc.vector.tensor_tensor(out_t[:], x2_sb[:, it, :],
                                                        tp[:].rearrange("p k f -> p (k f)"), op=ALU.add)
                                nc.sync.dma_start(out[it * 128:(it + 1) * 128, :], out_t[:])
                    rel(pool_h)
                    rel(pool_x2)

                for p in reversed(open_pools):
                    p.release()
                open_pools.clear()

    nc.compile()
    return nc


_CACHED_NC = None


def kernel(x, edge_bias, ln1_g, ln1_b, qkv_w, qkv_b, proj_w, proj_b,
           ln2_g, ln2_b, fc1_w, fc1_b, fc2_w, fc2_b):
    global _CACHED_NC
    if _CACHED_NC is None:
        _CACHED_NC = build_program()
    nc = _CACHED_NC

    w = _prep_weights(edge_bias, ln1_g, ln1_b, qkv_w, qkv_b, proj_w, proj_b,
                      ln2_g, ln2_b, fc1_w, fc1_b, fc2_w, fc2_b)
    x = np.asarray(x, np.float32)
    in_maps = []
    for c in range(N_CORES):
        m = dict(w)
        m["x"] = np.ascontiguousarray(x[c * BPC:(c + 1) * BPC].reshape(T, D))
        in_maps.append(m)

    res = run_bass_kernel_spmd(nc, in_maps, list(range(N_CORES)))
    outs = [res.results[c]["out"].reshape(BPC, S, D) for c in range(N_CORES)]
    return np.concatenate(outs, axis=0)


# revision 24
# speedup vs baseline: 2.6018x; 2.6018x over previous
"""Trainium2 Bass kernel for nn_AttentionBlock (B=32,S=512,D=768,H=12,FF=3072).

Strategy: pure data-parallel over batch. 8 NeuronCores x 4 batches each
(2048 tokens/core). Each core runs an identical Tile program:

  LN1 (token-major, per-partition stats) -> PE-transpose -> xn^T (feature-major)
  QKV: q^T,k^T feature-major; v token-major (with ones column for softmax sums)
  attention per (h,b): S^T = k^T.T @ q^T (transposed scores), P = exp(S^T)*exp(bias^T)
     (exp(edge_bias) precomputed on host; no max-subtraction needed: scores are tiny)
  o token-major = P^T.T @ [v|1]; per-partition softmax normalization
  o -> PE-transpose -> o^T -> proj -> transpose -> +x residual (fp32)
  LN2 -> transpose -> FFN (fc1+gelu -> h^T resident -> fc2) -> transpose -> +x2 -> out

All matmuls in bf16 (fp32 PSUM accumulation); LN stats, softmax denominators and
the residual stream stay fp32. LN gains/biases and the q-scale are folded into
the weights on the host; weights are pre-tiled on the host so every DMA is a
contiguous block.
"""

import numpy as np
import ml_dtypes

import concourse.bass as bass
import concourse.mybir as mybir
import concourse.tile as tile
from concourse import bacc
from concourse.bass_utils import run_bass_kernel_spmd
from concourse.masks import make_identity

AF = mybir.ActivationFunctionType
ALU = mybir.AluOpType
F32 = mybir.dt.float32
BF16 = mybir.dt.bfloat16

B, S, D, H = 32, 512, 768, 12
HD = D // H           # 64
FF = 4 * D            # 3072
EPS = 1e-5
N_CORES = 8
BPC = B // N_CORES    # 4 batches per core
T = BPC * S           # 2048 tokens per core
TT = T // 128         # 16 token tiles
DT = D // 128         # 6 feature tiles
FFT = FF // 128       # 24 ffn tiles
NCH = T // 512        # 4 column chunks of 512 tokens

bf16 = ml_dtypes.bfloat16


def _prep_weights(edge_bias, ln1_g, ln1_b, qkv_w, qkv_b, proj_w, proj_b,
                  ln2_g, ln2_b, fc1_w, fc1_b, fc2_w, fc2_b):
    """Host-side folding + retiling. Returns dict of np arrays for in_maps."""
    f32 = np.float32
    qkv_w = np.asarray(qkv_w, f32)
    qkv_b = np.asarray(qkv_b, f32)
    # fold LN1 affine into qkv
    w_eff = qkv_w * np.asarray(ln1_g, f32)[None, :]
    b_eff = qkv_b + qkv_w @ np.asarray(ln1_b, f32)
    # fold q scaling
    scale = HD ** -0.5
    w_eff[:D] *= scale
    b_eff = b_eff.copy()
    b_eff[:D] *= scale

    # q/k lhsT blocks: [12 fblocks, 128 dpart, 6 dt, 128 f] -> [12, 128, 768]
    wqk = w_eff[:2 * D].reshape(2 * DT, 128, DT, 128).transpose(0, 3, 2, 1)
    wqk = np.ascontiguousarray(wqk.reshape(2 * DT, 128, D), dtype=bf16)
    bqk = np.ascontiguousarray(b_eff[:2 * D].reshape(2 * DT, 128).T, dtype=f32)  # [128, 12]

    # v rhs blocks: [128 dpart, 6 dt, 768 vf] -> [128, 4608]
    wv = w_eff[2 * D:].reshape(D, DT, 128).transpose(2, 1, 0)
    wv = np.ascontiguousarray(wv.reshape(128, DT * D), dtype=bf16)
    bv = np.ascontiguousarray(b_eff[2 * D:].reshape(1, D), dtype=bf16)

    # proj lhsT blocks: [6 gblocks, 128 fpart, 6 ft, 128 g] -> [6, 128, 768]
    proj_w = np.asarray(proj_w, f32)
    wp = proj_w.reshape(DT, 128, DT, 128).transpose(0, 3, 2, 1)
    wp = np.ascontiguousarray(wp.reshape(DT, 128, D), dtype=bf16)
    bp = np.ascontiguousarray(np.asarray(proj_b, f32).reshape(DT, 128).T, dtype=f32)  # [128, 6]

    # fc1 (fold LN2): [24 ffblocks, 128 dpart, 6 dt, 128 ff] -> [24, 128, 768]
    fc1_w = np.asarray(fc1_w, f32)
    w1_eff = fc1_w * np.asarray(ln2_g, f32)[None, :]
    b1_eff = np.asarray(fc1_b, f32) + fc1_w @ np.asarray(ln2_b, f32)
    w1 = w1_eff.reshape(FFT, 128, DT, 128).transpose(0, 3, 2, 1)
    w1 = np.ascontiguousarray(w1.reshape(FFT, 128, D), dtype=bf16)
    b1 = np.ascontiguousarray(b1_eff.reshape(FFT, 128).T, dtype=f32)  # [128, 24]

    # fc2: [6 gblocks, 128 ffpart, 24 fft, 128 g] -> [6, 128, 3072]
    fc2_w = np.asarray(fc2_w, f32)
    w2 = fc2_w.reshape(DT, 128, FFT, 128).transpose(0, 3, 2, 1)
    w2 = np.ascontiguousarray(w2.reshape(DT, 128, FF), dtype=bf16)
    b2 = np.ascontiguousarray(np.asarray(fc2_b, f32).reshape(DT, 128).T, dtype=f32)  # [128, 6]

    # exp(edge_bias) transposed: [12, 128 ktpart, 4 kt, 512 qt] -> [12, 128, 2048]
    eb = np.exp(np.asarray(edge_bias, f32)).transpose(0, 2, 1)  # [H, kt, qt]
    eb = eb.reshape(H, NCH, 128, S).transpose(0, 2, 1, 3)
    eb = np.ascontiguousarray(eb.reshape(H, 128, NCH * S), dtype=bf16)

    return dict(wqk=wqk, bqk=bqk, wv=wv, bv=bv, wp=wp, bp=bp,
                w1=w1, b1=b1, w2=w2, b2=b2, expb=eb)


def _layernorm_tiles(nc, sb_small, x_ap, out_ap, apply_on_act=False):
    """Token-major LN core: out (bf16) = (x - mean)/sqrt(var+EPS).

    Stats on DVE (bn_stats/bn_aggr); apply is a single fused DVE tensor_scalar:
    (x + (-mean)) * rinv.
    """
    bn = sb_small.tile([128, 2, 6], F32, tag="bn")
    nc.vector.bn_stats(bn[:, 0, :], x_ap[:, 0:512])
    nc.vector.bn_stats(bn[:, 1, :], x_ap[:, 512:768])
    mv = sb_small.tile([128, 2], F32, tag="mv")
    nc.vector.bn_aggr(mv[:], bn[:])
    std = sb_small.tile([128, 1], F32, tag="std")
    nc.scalar.activation(std[:], mv[:, 1:2], AF.Sqrt, bias=EPS)
    rinv = sb_small.tile([128, 1], F32, tag="rinv")
    nc.vector.reciprocal(rinv[:], std[:])
    negmean = sb_small.tile([128, 1], F32, tag="negmean")
    nc.vector.tensor_scalar(negmean[:], mv[:, 0:1], -1.0, None, op0=ALU.mult)
    if apply_on_act:
        negmur = sb_small.tile([128, 1], F32, tag="negmur")
        nc.vector.tensor_tensor(negmur[:], negmean[:], rinv[:], op=ALU.mult)
        nc.scalar.activation(out_ap, x_ap, AF.Identity, bias=negmur[:], scale=rinv[:])
    else:
        nc.vector.tensor_scalar(out_ap, x_ap, negmean[:], rinv[:],
                                op0=ALU.add, op1=ALU.mult)


def build_program(reps=1, upto=6, av_variant='b'):
    nc = bacc.Bacc("TRN2", target_bir_lowering=False, debug=False)

    x = nc.dram_tensor("x", [T, D], F32, kind="ExternalInput").ap()
    wqk = nc.dram_tensor("wqk", [2 * DT, 128, D], BF16, kind="ExternalInput").ap()
    bqk = nc.dram_tensor("bqk", [128, 2 * DT], F32, kind="ExternalInput").ap()
    wv = nc.dram_tensor("wv", [128, DT * D], BF16, kind="ExternalInput").ap()
    bv = nc.dram_tensor("bv", [1, D], BF16, kind="ExternalInput").ap()
    wp = nc.dram_tensor("wp", [DT, 128, D], BF16, kind="ExternalInput").ap()
    bp = nc.dram_tensor("bp", [128, DT], F32, kind="ExternalInput").ap()
    w1 = nc.dram_tensor("w1", [FFT, 128, D], BF16, kind="ExternalInput").ap()
    b1 = nc.dram_tensor("b1", [128, FFT], F32, kind="ExternalInput").ap()
    w2 = nc.dram_tensor("w2", [DT, 128, FF], BF16, kind="ExternalInput").ap()
    b2 = nc.dram_tensor("b2", [128, DT], F32, kind="ExternalInput").ap()
    expb = nc.dram_tensor("expb", [H, 128, NCH * S], BF16, kind="ExternalInput").ap()
    out = nc.dram_tensor("out", [T, D], F32, kind="ExternalOutput").ap()

    with tile.TileContext(nc) as tc:
        with (
            tc.tile_pool(name="const", bufs=1) as constp,
            tc.tile_pool(name="small", bufs=4) as sb_small,
        ):
            ident = constp.tile([128, 128], BF16)
            make_identity(nc, ident[:])
            ones1 = constp.tile([1, 128], BF16)
            nc.vector.memset(ones1[:], 1.0)
            ones64f = constp.tile([1, HD], F32)
            nc.vector.memset(ones64f[:], 1.0)
            ones64r = constp.tile([1, HD], mybir.dt.float32r)
            nc.vector.tensor_copy(ones64r[:], ones64f[:])
            zero_c = constp.tile([128, 1], F32)
            nc.vector.memset(zero_c[:], 0.0)
            nc.const_aps.aps[(F32, 0.0)] = zero_c[:]
            eps_c = constp.tile([128, 1], F32)
            nc.vector.memset(eps_c[:], EPS)
            nc.const_aps.aps[(F32, EPS)] = eps_c[:]

            bqk_sb = constp.tile([128, 2 * DT], F32)
            nc.sync.dma_start(bqk_sb[:], bqk)
            bv_sb = constp.tile([1, D], BF16)
            nc.sync.dma_start(bv_sb[:], bv)
            bp_sb = constp.tile([128, DT], F32)
            nc.sync.dma_start(bp_sb[:], bp)
            b1_sb = constp.tile([128, FFT], F32)
            nc.sync.dma_start(b1_sb[:], b1)
            b2_sb = constp.tile([128, DT], F32)
            nc.sync.dma_start(b2_sb[:], b2)

            open_pools = []

            def pool(**kw):
                p = tc.alloc_tile_pool(**kw)
                open_pools.append(p)
                return p

            def rel(p):
                p.release()
                open_pools.remove(p)

            for _rep in range(reps):
                # ---------- Phase 1: LN1 + transpose -> xnT ----------
                pool_x = pool(name="xres", bufs=1)
                x_res = pool_x.tile([128, TT, D], F32)  # resident x, 48KB/part
                pool_xnT = pool(name="xnT", bufs=1)
                xnT = pool_xnT.tile([128, DT, T], BF16)  # 24KB/part
                with (
                    tc.tile_pool(name="p1", bufs=3) as p1,
                    tc.tile_pool(name="p1ps", bufs=2, space="PSUM") as p1ps,
                ):
                    for it in range(TT):
                        nc.sync.dma_start(x_res[:, it, :], x[it * 128:(it + 1) * 128, :])
                        xn_t = p1.tile([128, D], BF16, tag="xn")
                        _layernorm_tiles(nc, sb_small, x_res[:, it, :], xn_t[:])
                        tp = p1ps.tile([128, DT, 128], BF16, tag="tp")
                        for dt in range(DT):
                            nc.tensor.transpose(tp[:, dt, :], xn_t[:, dt * 128:(dt + 1) * 128], ident[:])
                        nc.vector.tensor_copy(xnT[:, :, it * 128:(it + 1) * 128], tp[:])

                # ---------- Phase 2+3 merged: QKV + attention ----------
                # v first; then per qk f-block pair (q_fb, k_fb) compute the
                # two heads' attention immediately — ACT exp and DVE softmax
                # work hides under the PE matmul stream.
                if upto >= 2:
                    pool_v = pool(name="vsb", bufs=1)
                    v_sb = pool_v.tile([128, TT, H, HD + 1], BF16)
                    nc.vector.memset(v_sb[:, :, :, HD], 1.0)
                    with (
                        tc.tile_pool(name="p2wv", bufs=1) as p2wv,
                        tc.tile_pool(name="p2vps", bufs=2, space="PSUM") as p2vps,
                    ):
                        wv_sb = p2wv.tile([128, DT, D], BF16, tag="wv")
                        nc.sync.dma_start(wv_sb[:], wv.rearrange("p (k f) -> p k f", k=DT))
                        for it in range(TT):
                            vps = p2vps.tile([128, D], F32, tag="vps")  # 2 banks
                            for kt in range(DT):
                                nc.tensor.matmul(vps[:, 0:512], xnT[:, kt, it * 128:(it + 1) * 128],
                                                 wv_sb[:, kt, 0:512], start=(kt == 0), stop=False)
                                nc.tensor.matmul(vps[:, 512:D], xnT[:, kt, it * 128:(it + 1) * 128],
                                                 wv_sb[:, kt, 512:D], start=(kt == 0), stop=False)
                            nc.tensor.matmul(vps[:, 0:512], ones1[:], bv_sb[:, 0:512],
                                             start=False, stop=True)
                            nc.tensor.matmul(vps[:, 512:D], ones1[:], bv_sb[:, 512:D],
                                             start=False, stop=True)
                            nc.scalar.activation(v_sb[:, it, :, 0:HD],
                                                 vps[:].rearrange("p (h d) -> p h d", h=H),
                                                 AF.Identity)

                    if upto >= 3:
                        pool_o = pool(name="o", bufs=1, side="right")
                        if av_variant == 'a':
                            oT = pool_o.tile([128, DT, T], BF16)  # feature-major o^T
                        else:
                            o_sb = pool_o.tile([128, TT, H, HD], BF16)  # token-major o
                    with (
                        tc.tile_pool(name="p2w", bufs=3) as p2w,
                        tc.tile_pool(name="p23qk", bufs=2) as p23qk,
                        tc.tile_pool(name="p3eb", bufs=3) as p3eb,
                        tc.tile_pool(name="p3p", bufs=3) as p3p,
                        tc.tile_pool(name="p2ps", bufs=1, space="PSUM") as p2ps,
                        tc.tile_pool(name="p3sps", bufs=2, space="PSUM") as p3sps,
                        tc.tile_pool(name="p3ops", bufs=2, space="PSUM") as p3ops,
                        tc.tile_pool(name="p3bcps", bufs=1, space="PSUM") as p3bcps,
                        tc.tile_pool(name="p3n", bufs=3) as p3n,
                    ):
                        for fp in range(DT):
                            # q f-block (fp) and k f-block (DT+fp) -> one pair tile
                            qk_t = p23qk.tile([128, 2, T], BF16, tag="qkpair")
                            for which, fb in ((0, fp), (1, DT + fp)):
                                wqk_t = p2w.tile([128, DT, 128], BF16, tag="wqk")
                                nc.sync.dma_start(wqk_t[:], wqk[fb].rearrange("p (k f) -> p k f", k=DT))
                                for chh in range(NCH):
                                    ps = p2ps.tile([128, 512], F32, tag="qkps")  # 1 bank
                                    for kt in range(DT):
                                        nc.tensor.matmul(ps[:],
                                                         wqk_t[:, kt, :],
                                                         xnT[:, kt, chh * 512:(chh + 1) * 512],
                                                         start=(kt == 0), stop=(kt == DT - 1))
                                    nc.scalar.activation(
                                        qk_t[:, which, chh * 512:(chh + 1) * 512],
                                        ps[:], AF.Identity, bias=bqk_sb[:, fb:fb + 1])
                            if upto < 3:
                                continue
                            # both heads of this f-block pair, scores emitted
                            # adjacently in disjoint PE row-groups (0-63 / 64-127)
                            ebs = []
                            for h in (2 * fp, 2 * fp + 1):
                                eb_tile = p3eb.tile([128, NCH, S], BF16, tag="eb")
                                nc.sync.dma_start(eb_tile[:],
                                                  expb[h].rearrange("p (j q) -> p j q", j=NCH))
                                ebs.append(eb_tile)
                            for b in range(BPC):
                                pTs = []
                                for hh in range(2):
                                    pT_h = p3p.tile([128, NCH, S], BF16, tag="pT")
                                    pTs.append(pT_h)
                                for jj in range(NCH // 2):
                                    sp0 = p3sps.tile([128, 2, S], F32, tag="s")
                                    sp1 = p3sps.tile([128, 2, S], F32, tag="s")
                                    for sub in range(2):
                                        j = jj * 2 + sub
                                        for hh, sp in ((0, sp0), (1, sp1)):
                                            prow = hh * HD
                                            q_ap = qk_t[prow:prow + HD, 0, b * S:(b + 1) * S]
                                            k_ap = qk_t[prow:prow + HD, 1,
                                                        b * S + j * 128: b * S + (j + 1) * 128]
                                            nc.tensor.matmul(sp[:, sub, :], k_ap, q_ap,
                                                             start=True, stop=True,
                                                             tile_position=(prow, 0))
                                    for hh, sp in ((0, sp0), (1, sp1)):
                                        nc.scalar.activation(
                                            pTs[hh][:, 2 * jj:2 * jj + 2, :], sp[:], AF.Exp)
                                        nc.vector.tensor_tensor(
                                            pTs[hh][:, 2 * jj:2 * jj + 2, :],
                                            pTs[hh][:, 2 * jj:2 * jj + 2, :],
                                            ebs[hh][:, 2 * jj:2 * jj + 2, :], op=ALU.mult)
                                for hh in range(2):
                                    h = 2 * fp + hh
                                    pT = pTs[hh]
                                    if av_variant == 'a':
                                        # o^T = [v|1].T @ P^T: out [65, 512], row 64 =
                                        # softmax denominators; feature-major directly.
                                        ops = p3ops.tile([HD + 1, S], F32, tag="o")
                                        for j in range(NCH):
                                            nc.tensor.matmul(ops[:],
                                                             v_sb[:, b * NCH + j, h, :],
                                                             pT[:, j, :],
                                                             start=(j == 0), stop=(j == NCH - 1))
                                        recr = p3n.tile([1, S], mybir.dt.float32r, tag="recr")
                                        with nc.allow_low_precision(
                                                reason="fp32r recip (~1e-4) is fine for softmax denominators"):
                                            nc.vector.reciprocal(recr[:], ops[HD:HD + 1, :])
                                        bcp = p3bcps.tile([HD, S], F32, tag="bc")
                                        nc.tensor.matmul(bcp[:], ones64r[:], recr[:],
                                                         start=True, stop=True)
                                        bcs = p3n.tile([HD, S], F32, tag="bcs")
                                        nc.scalar.activation(bcs[:], bcp[:], AF.Identity)
                                        prow2 = (h % 2) * HD
                                        nc.vector.tensor_tensor(
                                            oT[prow2:prow2 + HD, h // 2, b * S:(b + 1) * S],
                                            ops[0:HD, :], bcs[:], op=ALU.mult)
                                    else:
                                        ops = p3ops.tile([128, NCH, HD + 1], F32, tag="o")
                                        for qc in range(NCH):
                                            for j in range(NCH):
                                                nc.tensor.matmul(ops[:, qc, :],
                                                                 pT[:, j, qc * 128:(qc + 1) * 128],
                                                                 v_sb[:, b * NCH + j, h, :],
                                                                 start=(j == 0), stop=(j == NCH - 1))
                                        rec = sb_small.tile([128, NCH], F32, tag="rec")
                                        nc.vector.reciprocal(rec[:], ops[:, :, HD])
                                        for qc in range(NCH):
                                            nc.vector.tensor_scalar(o_sb[:, b * NCH + qc, h, :],
                                                                    ops[:, qc, 0:HD],
                                                                    rec[:, qc:qc + 1], None, op0=ALU.mult)
                    rel(pool_v)
                    rel(pool_xnT)

                # ---------- Phase 4: proj, residual -> x2 ----------
                if upto >= 4:
                    pool_p4 = pool(name="p4big", bufs=1)
                    with tc.tile_pool(name="p4", bufs=3) as p4:
                        if av_variant != 'a':
                            oT = pool_p4.tile([128, DT, T], BF16, tag="oT")
                            with tc.tile_pool(name="p4ps", bufs=2, space="PSUM") as p4ps:
                                for it in range(TT):
                                    tp = p4ps.tile([128, DT, 128], BF16, tag="otp")
                                    o_flat = o_sb[:, it, :, :].rearrange("p h d -> p (h d)")
                                    for dt in range(DT):
                                        nc.tensor.transpose(tp[:, dt, :],
                                                            o_flat[:, dt * 128:(dt + 1) * 128], ident[:])
                                    nc.vector.tensor_copy(oT[:, :, it * 128:(it + 1) * 128], tp[:])
                        pjT = pool_p4.tile([128, DT, T], BF16, tag="pjT")
                        with tc.tile_pool(name="p4pps", bufs=2, space="PSUM") as p4pps:
                            for gb in range(DT):
                                wp_t = p4.tile([128, DT, 128], BF16, tag="wp")
                                nc.sync.dma_start(wp_t[:], wp[gb].rearrange("p (k f) -> p k f", k=DT))
                                pps = p4pps.tile([128, T], F32, tag="pjps")
                                for kt in range(DT):
                                    for ch in range(NCH):
                                        nc.tensor.matmul(pps[:, ch * 512:(ch + 1) * 512],
                                                         wp_t[:, kt, :],
                                                         oT[:, kt, ch * 512:(ch + 1) * 512],
                                                         start=(kt == 0), stop=(kt == DT - 1))
                                nc.scalar.activation(pjT[:, gb, :], pps[:], AF.Identity,
                                                     bias=bp_sb[:, gb:gb + 1])
                        rel(pool_o)
                        pool_x2 = pool(name="x2", bufs=1, side="right")
                        x2_sb = pool_x2.tile([128, TT, D], F32)  # 48KB/part
                        with tc.tile_pool(name="p4tps", bufs=2, space="PSUM") as p4tps:
                            for it in range(TT):
                                tp = p4tps.tile([128, DT, 128], BF16, tag="rtp")
                                for dt in range(DT):
                                    nc.tensor.transpose(tp[:, dt, :],
                                                        pjT[:, dt, it * 128:(it + 1) * 128], ident[:])
                                nc.vector.tensor_tensor(x2_sb[:, it, :], x_res[:, it, :],
                                                        tp[:].rearrange("p k f -> p (k f)"), op=ALU.add)
                    rel(pool_p4)
                    rel(pool_x)

                # ---------- Phase 5: LN2 -> xn2T ----------
                if upto >= 5:
                    pool_xn2 = pool(name="xn2", bufs=1, side="right")
                    xn2T = pool_xn2.tile([128, DT, T], BF16)
                    with tc.tile_pool(name="p5", bufs=3) as p5, \
                         tc.tile_pool(name="p5ps", bufs=2, space="PSUM") as p5ps:
                        for it in range(TT):
                            xn2_t = p5.tile([128, D], BF16, tag="xn2")
                            _layernorm_tiles(nc, sb_small, x2_sb[:, it, :], xn2_t[:], apply_on_act=True)
                            tp = p5ps.tile([128, DT, 128], BF16, tag="tp2")
                            for dt in range(DT):
                                nc.tensor.transpose(tp[:, dt, :],
                                                    xn2_t[:, dt * 128:(dt + 1) * 128], ident[:])
                            nc.vector.tensor_copy(xn2T[:, :, it * 128:(it + 1) * 128], tp[:])

                # ---------- Phase 6: FFN + final residual ----------
                if upto >= 6:
                    pool_h = pool(name="hsb", bufs=1)
                    h_sb = pool_h.tile([128, FFT, T], BF16)  # 96KB/part
                    with tc.tile_pool(name="p6w1", bufs=2) as p6w1, \
                         tc.tile_pool(name="p6ps", bufs=2, space="PSUM") as p6ps:
                        for ffb in range(FFT):
                            w1_t = p6w1.tile([128, DT, 128], BF16, tag="w1")
                            nc.sync.dma_start(w1_t[:], w1[ffb].rearrange("p (k f) -> p k f", k=DT))
                            hps = p6ps.tile([128, T], F32, tag="hps")
                            for kt in range(DT):
                                for ch in range(NCH):
                                    nc.tensor.matmul(hps[:, ch * 512:(ch + 1) * 512],
                                                     w1_t[:, kt, :],
                                                     xn2T[:, kt, ch * 512:(ch + 1) * 512],
                                                     start=(kt == 0), stop=(kt == DT - 1))
                            nc.scalar.activation(h_sb[:, ffb, :], hps[:], AF.Gelu,
                                                 bias=b1_sb[:, ffb:ffb + 1])
                    rel(pool_xn2)

                    with (
                        tc.tile_pool(name="p6f", bufs=1) as p6f,
                        tc.tile_pool(name="p6w2", bufs=2) as p6w2,
                        tc.tile_pool(name="p6o", bufs=3) as p6o,
                    ):
                        f2T = p6f.tile([128, DT, T], BF16, tag="f2T")
                        # fc2 in 1024-wide halves, double-buffered psum
                        with tc.tile_pool(name="p6f2ps", bufs=2, space="PSUM") as p6f2ps:
                            for gb in range(DT):
                                w2_t = p6w2.tile([128, FFT, 128], BF16, tag="w2")
                                nc.sync.dma_start(w2_t[:], w2[gb].rearrange("p (k f) -> p k f", k=FFT))
                                for half in range(2):
                                    fps = p6f2ps.tile([128, T // 2], F32, tag="f2ps")  # 2 banks
                                    for kt in range(FFT):
                                        for ch in range(2):
                                            c = half * 2 + ch
                                            nc.tensor.matmul(fps[:, ch * 512:(ch + 1) * 512],
                                                             w2_t[:, kt, :],
                                                             h_sb[:, kt, c * 512:(c + 1) * 512],
                                                             start=(kt == 0), stop=(kt == FFT - 1))
                                    nc.scalar.activation(f2T[:, gb, half * 1024:(half + 1) * 1024],
                                                         fps[:], AF.Identity,
                                                         bias=b2_sb[:, gb:gb + 1])
                        with tc.tile_pool(name="p6tps", bufs=2, space="PSUM") as p6tps:
                            for it in range(TT):
                                tp = p6tps.tile([128, DT, 128], BF16, tag="ftp")
                                for dt in range(DT):
                                    nc.tensor.transpose(tp[:, dt, :],
                                                        f2T[:, dt, it * 128:(it + 1) * 128], ident[:])
                                out_t = p6o.tile([128, D], F32, tag="out")
                                nc.vector.tensor_tensor(out_t[:], x2_sb[:, it, :],
                                                        tp[:].rearrange("p k f -> p (k f)"), op=ALU.add)
                                nc.sync.dma_start(out[it * 128:(it + 1) * 128, :], out_t[:])
                    rel(pool_h)
                    rel(pool_x2)

                for p in reversed(open_pools):
                    p.release()
                open_pools.clear()

    nc.compile()
    return nc


_CACHED_NC = None


def kernel(x, edge_bias, ln1_g, ln1_b, qkv_w, qkv_b, proj_w, proj_b,
           ln2_g, ln2_b, fc1_w, fc1_b, fc2_w, fc2_b):
    global _CACHED_NC
    if _CACHED_NC is None:
        _CACHED_NC = build_program()
    nc = _CACHED_NC

    w = _prep_weights(edge_bias, ln1_g, ln1_b, qkv_w, qkv_b, proj_w, proj_b,
                      ln2_g, ln2_b, fc1_w, fc1_b, fc2_w, fc2_b)
    x = np.asarray(x, np.float32)
    in_maps = []
    for c in range(N_CORES):
        m = dict(w)
        m["x"] = np.ascontiguousarray(x[c * BPC:(c + 1) * BPC].reshape(T, D))
        in_maps.append(m)

    res = run_bass_kernel_spmd(nc, in_maps, list(range(N_CORES)))
    outs = [res.results[c]["out"].reshape(BPC, S, D) for c in range(N_CORES)]
    return np.concatenate(outs, axis=0)
